# revision 1
# baseline (speedup 1.0000x reference)
"""Multi-head attention (B=4, S=2048, C=768, H=8, HD=96) on 8 TRN2 NeuronCores.

Strategy: tensor-parallel by head — one head per core. All TensorEngine
matmuls run bf16 inputs with f32 PSUM accumulation.

  - QKV is computed directly transposed: qT/kT/vT [HD, tok] = W_chunk.T @ xT
    with the weight chunk stationary and 512-token moving operand (N=512),
    avoiding any PE transposes of x or q/k. xT and all weights are host
    pre-transposed bf16.
  - RoPE runs in the transposed layout: the pair-swap is one PE matmul with a
    host-provided [96,96] swap matrix; the sign lives in the sin table
    (s[2i] = -sin[2i]). Tables are host-transposed to [HD, S].
  - v is moved to k-major layout with 2-byte DMA transposes (XBAR), off PE.
  - Attention per (b, q-tile): scores.T [k,q] = kT.T @ qT on PE, exp on ACT
    (scale folded in; no max-subtraction needed: scores ~ N(0,1)), P.T (bf16)
    feeds PV directly: out.T [HD+1, q] = v_aug.T @ P.T, where v is augmented
    with a ones column so row HD accumulates the softmax denominator.
  - Normalization: reciprocal in token-parallel layout, then a stride-0-DMA
    broadcast of the reciprocal row from a DRAM bounce (no PE involved).
  - Token ownership is round-robin per batch (core h owns tokens
    [h*256,(h+1)*256) of every batch); each PAIR of batches shares one small
    AllToAll + one N=512 projection pass, so comm and projection overlap the
    following batches' attention and only the last pair's collective is on
    the critical tail.
  - PSUM pools are disjoint per pipeline phase so Tile's in-order slot
    granting never serializes one phase behind another.
"""

import numpy as np
from contextlib import ExitStack

import concourse.bass as bass
from concourse import bacc
import concourse.tile as tile
from concourse import mybir
from concourse.bass_utils import run_bass_kernel_spmd

B, S, C, H, HD = 4, 2048, 768, 8, 96
T = B * S            # 8192 tokens
NCORES = 8
TSLICE = T // NCORES  # 1024 tokens per core for the projection
BSLICE = S // NCORES  # 256 tokens per (core, batch)
KC = C // 128        # 6 contraction chunks of 128
F32 = mybir.dt.float32
BF16 = mybir.dt.bfloat16


def build_nc():
    nc = bacc.Bacc(None, num_devices=NCORES)

    xT = nc.declare_dram_parameter("xT", [C, T], BF16, isOutput=False)
    wqkvT = nc.declare_dram_parameter("wqkvT", [C, 3 * HD], BF16, isOutput=False)
    wprojT = nc.declare_dram_parameter("wprojT", [C, C], BF16, isOutput=False)
    cosT = nc.declare_dram_parameter("cosT", [HD, S], F32, isOutput=False)
    sT = nc.declare_dram_parameter("sT", [HD, S], F32, isOutput=False)
    biasd = nc.declare_dram_parameter("bias", [128, KC], F32, isOutput=False)
    outd = nc.declare_dram_parameter("out", [C, TSLICE], F32, isOutput=True)

    a2a_in = [nc.dram_tensor(f"a2a_in{p}", [C, 2 * BSLICE], BF16) for p in range(B // 2)]
    a2a_out = [nc.dram_tensor(f"a2a_out{p}", [C, 2 * BSLICE], BF16) for p in range(B // 2)]
    dnb = nc.dram_tensor("dnb", [1, 512], F32)  # denominator-reciprocal bounce

    SCALE = HD ** -0.5
    MULT = mybir.AluOpType.mult
    ADD = mybir.AluOpType.add
    EXP = mybir.ActivationFunctionType.Exp
    IDENT = mybir.ActivationFunctionType.Identity

    with tile.TileContext(nc, num_cores=NCORES) as tc, ExitStack() as ctx:
        const = ctx.enter_context(tc.tile_pool(name="const", bufs=1))
        xtp = ctx.enter_context(tc.tile_pool(name="xtp", bufs=2))
        rawp = ctx.enter_context(tc.tile_pool(name="rawp", bufs=4))
        ropep = ctx.enter_context(tc.tile_pool(name="ropep", bufs=4))
        Pp = ctx.enter_context(tc.tile_pool(name="Pp", bufs=4))
        nrm = ctx.enter_context(tc.tile_pool(name="nrm", bufs=4))
        rcp = ctx.enter_context(tc.tile_pool(name="rcp", bufs=3))
        yp = ctx.enter_context(tc.tile_pool(name="yp", bufs=3))
        agcp = ctx.enter_context(tc.tile_pool(name="agcp", bufs=2))

        # PSUM (8 banks), pools disjoint per phase:
        #   qkv accumulators 3 + scores 2 + PV acc 2 + proj 1
        psqkv = ctx.enter_context(tc.tile_pool(name="psqkv", bufs=3, space="PSUM"))
        pssc = ctx.enter_context(tc.tile_pool(name="pssc", bufs=2, space="PSUM"))
        psacc = ctx.enter_context(tc.tile_pool(name="psacc", bufs=2, space="PSUM"))
        pspy = ctx.enter_context(tc.tile_pool(name="pspy", bufs=1, space="PSUM"))

        # --- constants ---
        wq_sb = const.tile([128, KC, 3 * HD], BF16)
        nc.sync.dma_start(wq_sb, wqkvT.ap().rearrange("(kc p) n -> p kc n", p=128))
        wp_sb = const.tile([128, KC, C], BF16)
        nc.sync.dma_start(wp_sb, wprojT.ap().rearrange("(kc p) n -> p kc n", p=128))
        cosT_sb = const.tile([HD, S], F32)
        nc.sync.dma_start(cosT_sb, cosT.ap())
        sT_sb = const.tile([HD, S], F32)
        nc.sync.dma_start(sT_sb, sT.ap())
        bias_sb = const.tile([128, KC], F32)
        nc.sync.dma_start(bias_sb, biasd.ap())

        # persistent ping/pong per-batch q/k (transposed, channel-padded) and v
        qT = [const.tile([128, S], BF16, name=f"qT{i}") for i in range(2)]
        kT = [const.tile([128, S], BF16, name=f"kT{i}") for i in range(2)]
        vA = [const.tile([128, 16, 128], BF16, name=f"vA{i}") for i in range(2)]
        for i in range(2):
            nc.vector.memset(qT[i][HD:128, :], 0.0)
            nc.vector.memset(kT[i][HD:128, :], 0.0)


        vaug = [const.tile([128, 512], BF16, name=f"vaug{i}") for i in range(2)]
        for i in range(2):
            nc.vector.memset(vaug[i][HD:128, :], 0.0)
            nc.vector.memset(vaug[i][HD:HD + 1, :], 1.0)

        xTv = xT.ap().rearrange("(kc p) t -> p kc t", p=128)  # [128, KC, T]

        def do_proj(p):
            """Projection for this core's 512 tokens of batch pair p."""
            W = 2 * BSLICE
            agc = agcp.tile([128, KC, W], BF16)
            nc.sync.dma_start(
                agc, a2a_out[p].ap().rearrange("(kc p) t -> p kc t", p=128))
            if p == B // 2 - 1:
                # tail pair: qkv banks are idle — interleave 3 accumulators
                for kog in range(2):
                    pys = [psqkv.tile([128, W], F32, tag="qkv", name=f"pyt{i}")
                           for i in range(3)]
                    for kc in range(KC):
                        for i in range(3):
                            ko = kog * 3 + i
                            nc.tensor.matmul(
                                pys[i], wp_sb[:, kc, ko * 128:(ko + 1) * 128],
                                agc[:, kc, :],
                                start=(kc == 0), stop=(kc == KC - 1),
                            )
                    for i in range(3):
                        ko = kog * 3 + i
                        y_sb = yp.tile([128, W], F32)
                        nc.scalar.activation(
                            y_sb, pys[i], IDENT,
                            bias=bias_sb[:, ko:ko + 1], scale=1.0,
                        )
                        nc.sync.dma_start(
                            outd.ap()[ko * 128:(ko + 1) * 128, p * W:(p + 1) * W],
                            y_sb)
            else:
                for ko in range(KC):
                    y_sb = yp.tile([128, W], F32)
                    py = pspy.tile([128, W], F32)
                    for kc in range(KC):
                        nc.tensor.matmul(
                            py, wp_sb[:, kc, ko * 128:(ko + 1) * 128],
                            agc[:, kc, :],
                            start=(kc == 0), stop=(kc == KC - 1),
                        )
                    nc.scalar.activation(
                        y_sb, py, IDENT, bias=bias_sb[:, ko:ko + 1], scale=1.0,
                    )
                    nc.sync.dma_start(
                        outd.ap()[ko * 128:(ko + 1) * 128, p * W:(p + 1) * W],
                        y_sb)

        SWAPMASK = []
        for i in range(16):
            SWAPMASK += [2 * i + 1, 2 * i]

        def qkv_group(b, g):
            q_b, k_b, v_b = qT[b % 2], kT[b % 2], vA[b % 2]
            tok0 = b * S + g * 512
            seq = slice(g * 512, (g + 1) * 512)
            xts = []
            for kc in range(KC):
                xtc = xtp.tile([128, 512], BF16, tag=f"xtc{kc}", name=f"xtc{kc}")
                nc.sync.dma_start(xtc, xTv[:, kc, tok0:tok0 + 512])
                xts.append(xtc)
            # interleaved q/k/v accumulation across three PSUM banks so
            # consecutive matmuls never target the same bank
            ps = [psqkv.tile([HD, 512], F32, tag="qkv", name=f"qkvps{ti}") for ti in range(3)]
            for kc in range(KC):
                for ti in range(3):
                    nc.tensor.matmul(
                        ps[ti], wq_sb[:, kc, ti * HD:(ti + 1) * HD], xts[kc],
                        start=(kc == 0), stop=(kc == KC - 1),
                    )
            for ti, dstT in ((0, q_b), (1, k_b)):
                raw = rawp.tile([HD, 512], BF16, tag="raw")
                nc.vector.tensor_copy(out=raw, in_=ps[ti])
                rot = rawp.tile([HD, 512], BF16, tag="rot")
                nc.vector.stream_shuffle(rot, raw, SWAPMASK)
                t1 = ropep.tile([HD, 512], F32, tag="t1")
                nc.vector.tensor_tensor(t1, raw, cosT_sb[:, seq], MULT)
                t2 = ropep.tile([HD, 512], F32, tag="t2")
                nc.vector.tensor_tensor(t2, rot, sT_sb[:, seq], MULT)
                nc.vector.tensor_tensor(
                    dstT[0:HD, g * 512:(g + 1) * 512], t1, t2, ADD)
            vraw = vaug[g % 2]
            nc.vector.tensor_copy(out=vraw[0:HD, :], in_=ps[2])
            for c in range(4):
                nc.sync.dma_start(
                    out=v_b[:, 4 * g + c, :],
                    in_=vraw[:, c * 128:(c + 1) * 128],
                    transpose=True,
                )

        def attention_tile(b, qt):
            q_b, k_b, v_b = qT[b % 2], kT[b % 2], vA[b % 2]
            acc = psacc.tile([128, 512], F32)
            for kt in range(16):
                sc = pssc.tile([128, 512], F32)
                nc.tensor.matmul(
                    sc, k_b[:, kt * 128:(kt + 1) * 128],
                    q_b[:, qt * 512:(qt + 1) * 512],
                    start=True, stop=True,
                )
                Pt = Pp.tile([128, 512], BF16)
                nc.scalar.activation(Pt, sc, EXP, scale=SCALE)
                nc.tensor.matmul(
                    acc, v_b[:, kt, :], Pt,
                    start=(kt == 0), stop=(kt == 15),
                )
            # normalize: reciprocal of denominators (row HD of acc), then a
            # stride-0 DMA broadcast of the reciprocal row from DRAM
            dnrow = rcp.tile([1, 512], F32, tag="dnrow")
            nc.vector.tensor_copy(out=dnrow, in_=acc[HD:HD + 1, :])
            dn = rcp.tile([128, 4], F32, tag="dn")
            nc.sync.dma_start(dn, dnrow)
            rc = rcp.tile([128, 4], F32, tag="rc")
            nc.vector.reciprocal(rc, dn)
            nc.sync.dma_start(dnb.ap(), rc)
            bcast = nrm.tile([HD, 512], F32, tag="bcast")
            dnb_ap = dnb.ap()
            bcast_src = bass.AP(
                tensor=dnb_ap.tensor, offset=dnb_ap.offset,
                ap=[[0, HD]] + list(dnb_ap.ap)[1:],
            )
            nc.sync.dma_start(bcast, bcast_src)
            onorm = nrm.tile([HD, 512], BF16, tag="onorm")
            nc.vector.tensor_tensor(onorm, acc[0:HD, :], bcast, MULT)
            for half in range(2):
                j = 2 * qt + half
                co = (b % 2) * BSLICE
                nc.sync.dma_start(
                    a2a_in[b // 2].ap()[j * HD:(j + 1) * HD, co:co + BSLICE],
                    onorm[:, half * 256:(half + 1) * 256])

        # prologue: batch 0 qkv
        for g in range(4):
            qkv_group(0, g)
        for b in range(B):
            # attention(b) interleaved with qkv(b+1) in program order so the
            # scheduler spreads the next batch's prep across this batch
            for qt in range(4):
                attention_tile(b, qt)
                if b + 1 < B:
                    qkv_group(b + 1, qt)
            if b % 2 == 1:
                nc.gpsimd.collective_compute(
                    "AllToAll", mybir.AluOpType.bypass,
                    replica_groups=[list(range(NCORES))],
                    ins=[a2a_in[b // 2].ap().opt()],
                    outs=[a2a_out[b // 2].ap().opt()],
                )
                do_proj(b // 2)

    nc.compile()
    return nc


_NC_CACHE = None


def _get_nc():
    global _NC_CACHE
    if _NC_CACHE is None:
        _NC_CACHE = build_nc()
    return _NC_CACHE


def make_in_maps(x, cos, sin, Wqkv, Wproj, bproj):
    import ml_dtypes

    bf16 = ml_dtypes.bfloat16
    x = np.asarray(x, np.float32)
    cos = np.asarray(cos, np.float32)
    sin = np.asarray(sin, np.float32)
    Wqkv = np.asarray(Wqkv, np.float32)
    Wproj = np.asarray(Wproj, np.float32)
    bproj = np.asarray(bproj, np.float32)

    xT = np.ascontiguousarray(x.reshape(T, C).T.astype(bf16))  # [C, T] bf16
    wprojT = np.ascontiguousarray(Wproj.T.astype(bf16))        # [C_in, C_out]
    s = sin.copy()
    s[:, 0::2] = -sin[:, 0::2]
    cosT = np.ascontiguousarray(cos.T)                         # [HD, S] f32
    sT = np.ascontiguousarray(s.T)                             # [HD, S] f32
    bias2 = np.ascontiguousarray(bproj.reshape(KC, 128).T)     # [128, KC]

    in_maps = []
    for h in range(NCORES):
        wh = np.concatenate(
            [
                Wqkv[h * HD:(h + 1) * HD],                 # q rows
                Wqkv[C + h * HD:C + (h + 1) * HD],         # k rows
                Wqkv[2 * C + h * HD:2 * C + (h + 1) * HD], # v rows
            ],
            axis=0,
        )                                                  # [3*HD, C]
        wqkvT_h = np.ascontiguousarray(wh.T.astype(bf16))  # [C, 3*HD]
        in_maps.append({
            "xT": xT,
            "wqkvT": wqkvT_h,
            "wprojT": wprojT,
            "cosT": cosT,
            "sT": sT,
            "bias": bias2,
        })
    return in_maps


def assemble_output(results):
    # core h's out [C, 4*256]: columns b*256+i -> global token b*S + h*256 + i
    y = np.empty((T, C), np.float32)
    for h in range(NCORES):
        o = results[h]["out"].T  # [1024, C]
        for b in range(B):
            col = (b // 2) * 2 * BSLICE + (b % 2) * BSLICE
            y[b * S + h * BSLICE:b * S + (h + 1) * BSLICE] = \
                o[col:col + BSLICE]
    return y.reshape(B, S, C)


def kernel(x, cos, sin, Wqkv, Wproj, bproj, _trace=False, **run_kwargs):
    nc = _get_nc()
    in_maps = make_in_maps(x, cos, sin, Wqkv, Wproj, bproj)
    res = run_bass_kernel_spmd(
        nc, in_maps, core_ids=list(range(NCORES)), trace=_trace, **run_kwargs
    )
    out = assemble_output(res.results)
    kernel.last_results = res
    return out


if __name__ == "__main__":
    nc = build_nc()
    print("built OK, instructions:", len(nc.inst_map))



# revision 9
# speedup vs baseline: 1.0559x; 1.0559x over previous
"""Multi-head attention (B=4, S=2048, C=768, H=8, HD=96) on 8 TRN2 NeuronCores.

Strategy: tensor-parallel by head - one head per core. All TensorEngine
matmuls run bf16 inputs with f32 PSUM accumulation.

Pipeline design (v3):
  - Per-batch AllToAll for batches 0-2; batch 3 is split into two
    half-batch AllToAlls so the tail only waits for a 196KB collective
    plus a 36-matmul projection.
  - Projection for batch b runs as PE "filler" matmuls inside batch b+1's
    attention slots (qt=2,3); batch-3 halves project at the tail.
  - exp runs on [128,1024] PSUM tiles (2 banks), halving ACT instructions.
  - Explicit interleave per attention slot: [sc pair p+1][fillers][pv p]
    so the in-order Tensor queue never waits on the exp dependency.
  - x prefetch is a strict 2-steps-ahead queue over a 3-buffer pool so the
    GpSimd (SWDGE) queue never backs up behind slot-paced WAR waits -
    collective triggers on that queue fire immediately.
  - cos/sin tables bf16 (2x DVE RoPE multiplies); denominator reciprocal
    broadcast via a stride-0 DMA from a DRAM bounce (off the PE).
"""

import numpy as np
from contextlib import ExitStack

import concourse.bass as bass
from concourse import bacc
import concourse.tile as tile
from concourse import mybir
from concourse.bass_utils import run_bass_kernel_spmd

B, S, C, H, HD = 4, 2048, 768, 8, 96
T = B * S            # 8192 tokens
NCORES = 8
TSLICE = T // NCORES  # 1024 tokens per core for the projection
BSLICE = S // NCORES  # 256 tokens per (core, batch)
KC = C // 128        # 6 contraction chunks of 128
F32 = mybir.dt.float32
BF16 = mybir.dt.bfloat16

SCALE = HD ** -0.5
MULT = mybir.AluOpType.mult
ADD = mybir.AluOpType.add
EXP = mybir.ActivationFunctionType.Exp
IDENT = mybir.ActivationFunctionType.Identity

SWAPMASK = []
for i in range(16):
    SWAPMASK += [2 * i + 1, 2 * i]


def build_nc():
    nc = bacc.Bacc(None, num_devices=NCORES)

    xT = nc.declare_dram_parameter("xT", [C, T], BF16, isOutput=False)
    wqkvT = nc.declare_dram_parameter("wqkvT", [C, 3 * HD], BF16, isOutput=False)
    wprojT = nc.declare_dram_parameter("wprojT", [C, C], BF16, isOutput=False)
    cosT = nc.declare_dram_parameter("cosT", [HD, S], BF16, isOutput=False)
    sT = nc.declare_dram_parameter("sT", [HD, S], BF16, isOutput=False)
    biasd = nc.declare_dram_parameter("bias", [128, KC], F32, isOutput=False)
    outd = nc.declare_dram_parameter("out", [C, TSLICE], F32, isOutput=True)

    # batches 0-2: one [C, 256] AllToAll each; batch 3: two [C, 128] halves
    a2a_in = [nc.dram_tensor(f"a2a_in{b}", [C, BSLICE], BF16) for b in range(3)]
    a2a_out = [nc.dram_tensor(f"a2a_out{b}", [C, BSLICE], BF16) for b in range(3)]
    a2a3_in = [nc.dram_tensor(f"a2a3_in{h}", [C, 128], BF16) for h in range(2)]
    a2a3_out = [nc.dram_tensor(f"a2a3_out{h}", [C, 128], BF16) for h in range(2)]
    dnb = [nc.dram_tensor(f"dnb{i}", [1, 512], F32) for i in range(2)]

    with tile.TileContext(nc, num_cores=NCORES) as tc, ExitStack() as ctx:
        const = ctx.enter_context(tc.tile_pool(name="const", bufs=1))
        xtp = ctx.enter_context(tc.tile_pool(name="xtp", bufs=3))
        ropep = ctx.enter_context(tc.tile_pool(name="ropep", bufs=3))
        Pp = ctx.enter_context(tc.tile_pool(name="Pp", bufs=3))
        nrm = ctx.enter_context(tc.tile_pool(name="nrm", bufs=3))
        rcp = ctx.enter_context(tc.tile_pool(name="rcp", bufs=3))
        yp = ctx.enter_context(tc.tile_pool(name="yp", bufs=3))
        agcp = ctx.enter_context(tc.tile_pool(name="agcp", bufs=2))

        # PSUM (8 banks): scores 2x2 + PV acc 2 + qkv/proj 2
        pssc = ctx.enter_context(tc.tile_pool(name="pssc", bufs=2, space="PSUM"))
        psacc = ctx.enter_context(tc.tile_pool(name="psacc", bufs=2, space="PSUM"))
        psqkv = ctx.enter_context(tc.tile_pool(name="psqkv", bufs=2, space="PSUM"))

        # --- constants (wq first so the PE can start ASAP) ---
        wq_sb = const.tile([128, KC, 3 * HD], BF16)
        nc.sync.dma_start(wq_sb, wqkvT.ap().rearrange("(kc p) n -> p kc n", p=128))
        cosT_sb = const.tile([HD, S], BF16)
        nc.sync.dma_start(cosT_sb, cosT.ap())
        sT_sb = const.tile([HD, S], BF16)
        nc.sync.dma_start(sT_sb, sT.ap())
        bias_sb = const.tile([128, KC], F32)
        nc.sync.dma_start(bias_sb, biasd.ap())
        wp_sb = const.tile([128, KC, C], BF16)
        nc.sync.dma_start(wp_sb, wprojT.ap().rearrange("(kc p) n -> p kc n", p=128))

        # persistent ping/pong per-batch q/k (transposed) and token-major v
        qT = [const.tile([HD, S], BF16, name=f"qT{i}") for i in range(2)]
        kT = [const.tile([HD, S], BF16, name=f"kT{i}") for i in range(2)]
        vA = [const.tile([128, 16, 128], BF16, name=f"vA{i}") for i in range(2)]

        # staging buffer for v before its DMA transpose: rows 96..127 are
        # (ones, zeros...) so the transposed tiles carry the denominator
        # column at position 96
        vaug = [const.tile([128, 512], BF16, name=f"vaug{i}") for i in range(2)]
        for i in range(2):
            nc.vector.memset(vaug[i][HD:128, :], 0.0)
            nc.vector.memset(vaug[i][HD:HD + 1, :], 1.0)

        xTv = xT.ap().rearrange("(kc p) t -> p kc t", p=128)  # [128, KC, T]

        # ---------- emission helpers ----------
        state = {"xtc": {}, "agc": {}}
        CONS = [(b, g) for b in range(B) for g in range(4)]  # consumption order

        def load_x_group(idx, sync=False):
            """Prefetch one 512-token x group (consumption-order index)."""
            if idx >= len(CONS):
                return
            b, g = CONS[idx]
            tok0 = b * S + g * 512
            xtc = xtp.tile([128, KC, 512], BF16, tag="xtc", name="xtc")
            eng = nc.sync if sync else nc.gpsimd
            eng.dma_start(xtc, xTv[:, :, tok0:tok0 + 512])
            state["xtc"][(b, g)] = xtc

        def qkv_fillers(b, g):
            """Closures, each emitting one PE matmul of the qkv computation
            for (batch b, 512-token group g); drains are emitted inline by
            the closure that finishes each accumulation."""
            xtc = state["xtc"].pop((b, g))
            seq = slice(g * 512, (g + 1) * 512)
            ps_tiles = {}

            def drain_qk(ps, dstT):
                raw = ropep.tile([HD, 512], BF16, tag="raw", name="raw")
                nc.vector.tensor_copy(out=raw, in_=ps[0:HD, :])
                rot = ropep.tile([HD, 512], BF16, tag="rot", name="rot")
                nc.vector.stream_shuffle(rot, raw, SWAPMASK)
                t1 = ropep.tile([HD, 512], BF16, tag="t1", name="t1")
                nc.vector.tensor_tensor(t1, raw, cosT_sb[:, seq], MULT)
                t2 = ropep.tile([HD, 512], BF16, tag="t2", name="t2")
                nc.vector.tensor_tensor(t2, rot, sT_sb[:, seq], MULT)
                nc.vector.tensor_tensor(dstT[:, seq], t1, t2, ADD)

            def drain_v(ps):
                vraw = vaug[g % 2]
                nc.vector.tensor_copy(out=vraw[0:HD, :], in_=ps[0:HD, :])
                for c in range(4):
                    nc.sync.dma_start(
                        out=vA[b % 2][:, 4 * g + c, :],
                        in_=vraw[:, c * 128:(c + 1) * 128],
                        transpose=True,
                    )

            def mk(ti, kc):
                def emit():
                    if kc == 0:
                        ps_tiles[ti] = psqkv.tile(
                            [128, 512], F32, tag="qkv", name="qkvps")
                    ps = ps_tiles[ti]
                    nc.tensor.matmul(
                        ps[0:HD, :], wq_sb[:, kc, ti * HD:(ti + 1) * HD],
                        xtc[:, kc, :],
                        start=(kc == 0), stop=(kc == KC - 1),
                    )
                    if kc == KC - 1:
                        if ti == 0:
                            drain_qk(ps, qT[b % 2])
                        elif ti == 1:
                            drain_qk(ps, kT[b % 2])
                        else:
                            drain_v(ps)
                return emit

            return [mk(ti, kc) for ti in range(3) for kc in range(KC)]

        def proj_fillers(key, chunks, ncols, colbase):
            """Closures for proj chunks (6 matmuls + drain each). `key`
            selects the gathered buffer; output columns [colbase,
            colbase+ncols)."""
            fillers = []
            for ko in chunks:
                def mk(ko):
                    py_ref = {}

                    def emit_mm(kc):
                        if kc == 0:
                            py_ref["py"] = psqkv.tile(
                                [128, 512], F32, tag="qkv", name="pyps")
                        py = py_ref["py"]
                        nc.tensor.matmul(
                            py[:, 0:ncols],
                            wp_sb[:, kc, ko * 128:(ko + 1) * 128],
                            state["agc"][key][:, kc, :],
                            start=(kc == 0), stop=(kc == KC - 1),
                        )
                        if kc == KC - 1:
                            y = yp.tile([128, ncols], F32, tag="y", name="y")
                            nc.scalar.activation(
                                y, py[:, 0:ncols], IDENT,
                                bias=bias_sb[:, ko:ko + 1], scale=1.0,
                            )
                            nc.sync.dma_start(
                                outd.ap()[ko * 128:(ko + 1) * 128,
                                          colbase:colbase + ncols],
                                y)
                    return [lambda kc=kc: emit_mm(kc) for kc in range(KC)]
                fillers += mk(ko)
            return fillers

        def trigger_a2a(ins, outs, key, ncols):
            nc.gpsimd.collective_compute(
                "AllToAll", mybir.AluOpType.bypass,
                replica_groups=[list(range(NCORES))],
                ins=[ins.ap().opt()],
                outs=[outs.ap().opt()],
            )
            agc = agcp.tile([128, KC, ncols], BF16, tag=f"agc{ncols}",
                            name="agc")
            nc.gpsimd.dma_start(
                agc, outs.ap().rearrange("(kc p) t -> p kc t", p=128))
            state["agc"][key] = agc

        norm_b = {"pending": None}

        def emit_norm_b():
            """Part B of the previous slot's normalize: the broadcast-mult,
            the a2a staging writes, and (on batch boundaries) the collective
            trigger + gather prefetch."""
            if norm_b["pending"] is None:
                return
            b, qt, acc, bcast = norm_b["pending"]
            norm_b["pending"] = None
            onorm = nrm.tile([HD, 512], BF16, tag="onorm", name="onorm")
            nc.vector.tensor_tensor(onorm, acc[0:HD, :], bcast, MULT)
            if b < 3:
                for half in range(2):
                    j = 2 * qt + half
                    nc.sync.dma_start(
                        a2a_in[b].ap()[j * HD:(j + 1) * HD, :],
                        onorm[:, half * 256:(half + 1) * 256])
                if qt == 3:
                    trigger_a2a(a2a_in[b], a2a_out[b], b, BSLICE)
            else:
                # batch 3: owner j holds tokens [j*128,(j+1)*128) per half
                bh = qt // 2
                for c in range(4):
                    j = 4 * (qt % 2) + c
                    nc.sync.dma_start(
                        a2a3_in[bh].ap()[j * HD:(j + 1) * HD, :],
                        onorm[:, c * 128:(c + 1) * 128])
                if qt % 2 == 1:
                    trigger_a2a(a2a3_in[bh], a2a3_out[bh], f"3{bh}", 128)

        def attention_slot(b, qt, fillers):
            """One attention tile (512 q tokens, 16 k tiles as 8 pairs) with
            filler matmuls interleaved so the PE stays dense."""
            q_b, k_b, v_b = qT[b % 2], kT[b % 2], vA[b % 2]
            fill = list(fillers)
            fi = 0
            nfill = len(fill)
            acc = psacc.tile([128, 512], F32, name="acc")
            Pt_t = [None] * 8

            def emit_sc(p):
                sc = pssc.tile([128, 1024], F32, tag="sc", name="sc")
                for h in range(2):
                    kt = 2 * p + h
                    nc.tensor.matmul(
                        sc[:, h * 512:(h + 1) * 512],
                        k_b[:, kt * 128:(kt + 1) * 128],
                        q_b[:, qt * 512:(qt + 1) * 512],
                        start=True, stop=True,
                    )
                Pt = Pp.tile([128, 1024], BF16, tag="P", name="Pt")
                Pt_t[p] = Pt
                nc.scalar.activation(Pt, sc, EXP, scale=SCALE)

            def emit_pv(p):
                Pt = Pt_t[p]
                for h in range(2):
                    kt = 2 * p + h
                    nc.tensor.matmul(
                        acc, v_b[:, kt, :], Pt[:, h * 512:(h + 1) * 512],
                        start=(kt == 0), stop=(kt == 15),
                    )

            # interleave: sc(p+1) ... fillers ... pv(p)
            emit_sc(0)
            for p in range(8):
                if p + 1 < 8:
                    emit_sc(p + 1)
                # spread fillers evenly over the 8 pair slots
                ntake = (nfill * (p + 1)) // 8 - fi
                for _ in range(ntake):
                    fill[fi]()
                    fi += 1
                if p == 2:
                    # previous slot's normalize part B: by now its broadcast
                    # DMA has landed, so the DVE queue won't block on it
                    emit_norm_b()
                emit_pv(p)

            # normalize part A: denominators -> reciprocal -> DRAM bounce ->
            # stride-0 broadcast (no PE involved)
            dnrow = rcp.tile([1, 512], F32, tag="dnrow", name="dnrow")
            nc.vector.tensor_copy(out=dnrow, in_=acc[HD:HD + 1, :])
            dn = rcp.tile([128, 4], F32, tag="dn", name="dn")
            nc.sync.dma_start(dn, dnrow)
            rc = rcp.tile([128, 4], F32, tag="rc", name="rc")
            nc.vector.reciprocal(rc, dn)
            bounce = dnb[qt % 2]
            nc.sync.dma_start(bounce.ap(), rc)
            bcast = nrm.tile([HD, 512], F32, tag="bcast", name="bcast")
            b_ap = bounce.ap()
            bcast_src = bass.AP(
                tensor=b_ap.tensor, offset=b_ap.offset,
                ap=[[0, HD]] + list(b_ap.ap)[1:],
            )
            nc.sync.dma_start(bcast, bcast_src)
            norm_b["pending"] = (b, qt, acc, bcast)

        # ---------- main schedule ----------
        # x prefetch: strict consumption-order queue, 2 steps ahead
        load_x_group(0, sync=True)
        load_x_group(1, sync=True)
        # prologue: qkv(0) dense
        for g in range(4):
            for f in qkv_fillers(0, g):
                f()
            load_x_group(g + 2)

        for b in range(B):
            for qt in range(4):
                step = 4 + b * 4 + qt
                fillers = []
                if b + 1 < B:
                    fillers += qkv_fillers(b + 1, qt)
                if b >= 1 and qt >= 2:
                    # 3 proj chunks of the previous batch per late slot
                    chunks = [0, 1, 2] if qt == 2 else [3, 4, 5]
                    fillers += proj_fillers(
                        b - 1, chunks, BSLICE, (b - 1) * BSLICE)
                attention_slot(b, qt, fillers)
                load_x_group(step + 2)
        emit_norm_b()
        # tail: batch-3 projections (half a overlaps half b's collective)
        for f in proj_fillers("30", [0, 1, 2, 3, 4, 5], 128, 3 * BSLICE):
            f()
        for f in proj_fillers("31", [0, 1, 2, 3, 4, 5], 128, 3 * BSLICE + 128):
            f()

    nc.compile()
    return nc


_NC_CACHE = None


def _get_nc():
    global _NC_CACHE
    if _NC_CACHE is None:
        _NC_CACHE = build_nc()
    return _NC_CACHE


def make_in_maps(x, cos, sin, Wqkv, Wproj, bproj):
    import ml_dtypes

    bf16 = ml_dtypes.bfloat16
    x = np.asarray(x, np.float32)
    cos = np.asarray(cos, np.float32)
    sin = np.asarray(sin, np.float32)
    Wqkv = np.asarray(Wqkv, np.float32)
    Wproj = np.asarray(Wproj, np.float32)
    bproj = np.asarray(bproj, np.float32)

    xT = np.ascontiguousarray(x.reshape(T, C).T.astype(bf16))  # [C, T] bf16
    wprojT = np.ascontiguousarray(Wproj.T.astype(bf16))        # [C_in, C_out]
    s = sin.copy()
    s[:, 0::2] = -sin[:, 0::2]
    cosT = np.ascontiguousarray(cos.T.astype(bf16))            # [HD, S] bf16
    sT = np.ascontiguousarray(s.T.astype(bf16))                # [HD, S] bf16
    bias2 = np.ascontiguousarray(bproj.reshape(KC, 128).T)     # [128, KC]

    in_maps = []
    for h in range(NCORES):
        wh = np.concatenate(
            [
                Wqkv[h * HD:(h + 1) * HD],                 # q rows
                Wqkv[C + h * HD:C + (h + 1) * HD],         # k rows
                Wqkv[2 * C + h * HD:2 * C + (h + 1) * HD], # v rows
            ],
            axis=0,
        )                                                  # [3*HD, C]
        wqkvT_h = np.ascontiguousarray(wh.T.astype(bf16))  # [C, 3*HD]
        in_maps.append({
            "xT": xT,
            "wqkvT": wqkvT_h,
            "wprojT": wprojT,
            "cosT": cosT,
            "sT": sT,
            "bias": bias2,
        })
    return in_maps


def assemble_output(results):
    # batches 0-2: core h's out cols b*256+t <-> token b*S + h*256 + t
    # batch 3: cols 768+t <-> token 3*S + h*128 + t (half a),
    #          cols 896+t <-> token 3*S + 1024 + h*128 + t (half b)
    y = np.empty((T, C), np.float32)
    for h in range(NCORES):
        o = results[h]["out"].T  # [1024, C]
        for b in range(3):
            y[b * S + h * BSLICE:b * S + (h + 1) * BSLICE] = \
                o[b * BSLICE:(b + 1) * BSLICE]
        y[3 * S + h * 128:3 * S + (h + 1) * 128] = o[768:896]
        y[3 * S + 1024 + h * 128:3 * S + 1024 + (h + 1) * 128] = o[896:1024]
    return y.reshape(B, S, C)


def kernel(x, cos, sin, Wqkv, Wproj, bproj, _trace=False, **run_kwargs):
    nc = _get_nc()
    in_maps = make_in_maps(x, cos, sin, Wqkv, Wproj, bproj)
    res = run_bass_kernel_spmd(
        nc, in_maps, core_ids=list(range(NCORES)), trace=_trace, **run_kwargs
    )
    out = assemble_output(res.results)
    kernel.last_results = res
    return out


if __name__ == "__main__":
    nc = build_nc()
    print("built OK, instructions:", len(nc.inst_map))


# revision 11
# speedup vs baseline: 1.0691x; 1.0125x over previous
"""Multi-head attention (B=4, S=2048, C=768, H=8, HD=96) on 8 TRN2 NeuronCores.

Strategy: tensor-parallel by head - one head per core. All TensorEngine
matmuls run bf16 inputs with f32 PSUM accumulation.

Pipeline design (v3):
  - Per-batch AllToAll for batches 0-2; batch 3 is split into two
    half-batch AllToAlls so the tail only waits for a 196KB collective
    plus a 36-matmul projection.
  - Projection for batch b runs as PE "filler" matmuls inside batch b+1's
    attention slots (qt=2,3); batch-3 halves project at the tail.
  - exp runs on [128,1024] PSUM tiles (2 banks), halving ACT instructions.
  - Explicit interleave per attention slot: [sc pair p+1][fillers][pv p]
    so the in-order Tensor queue never waits on the exp dependency.
  - x prefetch is a strict 2-steps-ahead queue over a 3-buffer pool so the
    GpSimd (SWDGE) queue never backs up behind slot-paced WAR waits -
    collective triggers on that queue fire immediately.
  - cos/sin tables bf16 (2x DVE RoPE multiplies); denominator reciprocal
    broadcast via a stride-0 DMA from a DRAM bounce (off the PE).
"""

import numpy as np
from contextlib import ExitStack

import concourse.bass as bass
from concourse import bacc
import concourse.tile as tile
from concourse import mybir
from concourse.bass_utils import run_bass_kernel_spmd

B, S, C, H, HD = 4, 2048, 768, 8, 96
T = B * S            # 8192 tokens
NCORES = 8
TSLICE = T // NCORES  # 1024 tokens per core for the projection
BSLICE = S // NCORES  # 256 tokens per (core, batch)
KC = C // 128        # 6 contraction chunks of 128
F32 = mybir.dt.float32
BF16 = mybir.dt.bfloat16

SCALE = HD ** -0.5
MULT = mybir.AluOpType.mult
ADD = mybir.AluOpType.add
EXP = mybir.ActivationFunctionType.Exp
IDENT = mybir.ActivationFunctionType.Identity

SWAPMASK = []
for i in range(16):
    SWAPMASK += [2 * i + 1, 2 * i]


def build_nc():
    nc = bacc.Bacc(None, num_devices=NCORES)

    xT = nc.declare_dram_parameter("xT", [C, T], BF16, isOutput=False)
    wqkvT = nc.declare_dram_parameter("wqkvT", [C, 3 * HD], BF16, isOutput=False)
    wprojT = nc.declare_dram_parameter("wprojT", [C, C], BF16, isOutput=False)
    cosT = nc.declare_dram_parameter("cosT", [HD, S], BF16, isOutput=False)
    sT = nc.declare_dram_parameter("sT", [HD, S], BF16, isOutput=False)
    biasd = nc.declare_dram_parameter("bias", [128, KC], F32, isOutput=False)
    outd = nc.declare_dram_parameter("out", [C, TSLICE], F32, isOutput=True)

    # batches 0-2: one [C, 256] AllToAll each; batch 3: two [C, 128] halves
    a2a_in = [nc.dram_tensor(f"a2a_in{b}", [C, BSLICE], BF16) for b in range(3)]
    a2a_out = [nc.dram_tensor(f"a2a_out{b}", [C, BSLICE], BF16) for b in range(3)]
    a2a3_in = [nc.dram_tensor(f"a2a3_in{h}", [C, 128], BF16) for h in range(2)]
    a2a3_out = [nc.dram_tensor(f"a2a3_out{h}", [C, 128], BF16) for h in range(2)]
    dnb = [nc.dram_tensor(f"dnb{i}", [1, 512], F32) for i in range(2)]

    with tile.TileContext(nc, num_cores=NCORES) as tc, ExitStack() as ctx:
        const = ctx.enter_context(tc.tile_pool(name="const", bufs=1))
        xtp = ctx.enter_context(tc.tile_pool(name="xtp", bufs=3))
        ropep = ctx.enter_context(tc.tile_pool(name="ropep", bufs=3))
        Pp = ctx.enter_context(tc.tile_pool(name="Pp", bufs=3))
        nrm = ctx.enter_context(tc.tile_pool(name="nrm", bufs=3))
        rcp = ctx.enter_context(tc.tile_pool(name="rcp", bufs=3))
        yp = ctx.enter_context(tc.tile_pool(name="yp", bufs=3))
        agcp = ctx.enter_context(tc.tile_pool(name="agcp", bufs=2))

        # PSUM (8 banks): scores 2x2 + PV acc 2 + qkv/proj 2
        pssc = ctx.enter_context(tc.tile_pool(name="pssc", bufs=2, space="PSUM"))
        psacc = ctx.enter_context(tc.tile_pool(name="psacc", bufs=2, space="PSUM"))
        psqkv = ctx.enter_context(tc.tile_pool(name="psqkv", bufs=2, space="PSUM"))

        # --- constants (wq first so the PE can start ASAP) ---
        wq_sb = const.tile([128, KC, 3 * HD], BF16)
        nc.sync.dma_start(wq_sb, wqkvT.ap().rearrange("(kc p) n -> p kc n", p=128))
        cosT_sb = const.tile([HD, S], BF16)
        nc.sync.dma_start(cosT_sb, cosT.ap())
        sT_sb = const.tile([HD, S], BF16)
        nc.sync.dma_start(sT_sb, sT.ap())
        bias_sb = const.tile([128, KC], F32)
        nc.sync.dma_start(bias_sb, biasd.ap())
        wp_sb = const.tile([128, KC, C], BF16)
        nc.sync.dma_start(wp_sb, wprojT.ap().rearrange("(kc p) n -> p kc n", p=128))

        # persistent ping/pong per-batch q/k (transposed) and token-major v
        qT = [const.tile([HD, S], BF16, name=f"qT{i}") for i in range(2)]
        kT = [const.tile([HD, S], BF16, name=f"kT{i}") for i in range(2)]
        vA = [const.tile([128, 16, 128], BF16, name=f"vA{i}") for i in range(2)]

        # staging buffer for v before its DMA transpose: rows 96..127 are
        # (ones, zeros...) so the transposed tiles carry the denominator
        # column at position 96
        vaug = [const.tile([128, 512], BF16, name=f"vaug{i}") for i in range(2)]
        for i in range(2):
            nc.vector.memset(vaug[i][HD:128, :], 0.0)
            nc.vector.memset(vaug[i][HD:HD + 1, :], 1.0)

        xTv = xT.ap().rearrange("(kc p) t -> p kc t", p=128)  # [128, KC, T]

        # ---------- emission helpers ----------
        state = {"xtc": {}, "agc": {}}
        CONS = [(b, g) for b in range(B) for g in range(4)]  # consumption order

        def load_x_group(idx, sync=False):
            """Prefetch one 512-token x group (consumption-order index)."""
            if idx >= len(CONS):
                return
            b, g = CONS[idx]
            tok0 = b * S + g * 512
            xtc = xtp.tile([128, KC, 512], BF16, tag="xtc", name="xtc")
            eng = nc.sync if sync else nc.gpsimd
            eng.dma_start(xtc, xTv[:, :, tok0:tok0 + 512])
            state["xtc"][(b, g)] = xtc

        def qkv_fillers(b, g):
            """Closures, each emitting one PE matmul of the qkv computation
            for (batch b, 512-token group g); drains are emitted inline by
            the closure that finishes each accumulation."""
            xtc = state["xtc"].pop((b, g))
            seq = slice(g * 512, (g + 1) * 512)
            ps_tiles = {}

            def drain_qk(ps, dstT):
                raw = ropep.tile([HD, 512], BF16, tag="raw", name="raw")
                nc.vector.tensor_copy(out=raw, in_=ps[0:HD, :])
                rot = ropep.tile([HD, 512], BF16, tag="rot", name="rot")
                nc.vector.stream_shuffle(rot, raw, SWAPMASK)
                t1 = ropep.tile([HD, 512], BF16, tag="t1", name="t1")
                nc.vector.tensor_tensor(t1, raw, cosT_sb[:, seq], MULT)
                t2 = ropep.tile([HD, 512], BF16, tag="t2", name="t2")
                nc.vector.tensor_tensor(t2, rot, sT_sb[:, seq], MULT)
                nc.vector.tensor_tensor(dstT[:, seq], t1, t2, ADD)

            def drain_v(ps):
                # cast on the Scalar engine: the DVE queue is deep in RoPE
                # work here, and the DMA transposes (Sync queue) must not
                # wait for it - PV matmuls depend on them
                vraw = vaug[g % 2]
                nc.scalar.copy(out=vraw[0:HD, :], in_=ps[0:HD, :])
                for c in range(4):
                    nc.sync.dma_start(
                        out=vA[b % 2][:, 4 * g + c, :],
                        in_=vraw[:, c * 128:(c + 1) * 128],
                        transpose=True,
                    )

            def mk(ti, kc):
                def emit():
                    if kc == 0:
                        ps_tiles[ti] = psqkv.tile(
                            [128, 512], F32, tag="qkv", name="qkvps")
                    ps = ps_tiles[ti]
                    nc.tensor.matmul(
                        ps[0:HD, :], wq_sb[:, kc, ti * HD:(ti + 1) * HD],
                        xtc[:, kc, :],
                        start=(kc == 0), stop=(kc == KC - 1),
                    )
                    if kc == KC - 1:
                        if ti == 0:
                            drain_qk(ps, qT[b % 2])
                        elif ti == 1:
                            drain_qk(ps, kT[b % 2])
                        else:
                            drain_v(ps)
                return emit

            # v first: its cast + DMA transposes are on the critical path
            # for the next batch's PV matmuls; RoPE drains can trail
            return [mk(ti, kc) for ti in (2, 0, 1) for kc in range(KC)]

        def proj_fillers(key, chunks, ncols, colbase):
            """Closures for proj chunks (6 matmuls + drain each). `key`
            selects the gathered buffer; output columns [colbase,
            colbase+ncols)."""
            fillers = []
            for ko in chunks:
                def mk(ko):
                    py_ref = {}

                    def emit_mm(kc):
                        if kc == 0:
                            py_ref["py"] = psqkv.tile(
                                [128, 512], F32, tag="qkv", name="pyps")
                        py = py_ref["py"]
                        nc.tensor.matmul(
                            py[:, 0:ncols],
                            wp_sb[:, kc, ko * 128:(ko + 1) * 128],
                            state["agc"][key][:, kc, :],
                            start=(kc == 0), stop=(kc == KC - 1),
                        )
                        if kc == KC - 1:
                            y = yp.tile([128, ncols], F32, tag="y", name="y")
                            nc.scalar.activation(
                                y, py[:, 0:ncols], IDENT,
                                bias=bias_sb[:, ko:ko + 1], scale=1.0,
                            )
                            nc.sync.dma_start(
                                outd.ap()[ko * 128:(ko + 1) * 128,
                                          colbase:colbase + ncols],
                                y)
                    return [lambda kc=kc: emit_mm(kc) for kc in range(KC)]
                fillers += mk(ko)
            return fillers

        def trigger_a2a(ins, outs, key, ncols):
            nc.gpsimd.collective_compute(
                "AllToAll", mybir.AluOpType.bypass,
                replica_groups=[list(range(NCORES))],
                ins=[ins.ap().opt()],
                outs=[outs.ap().opt()],
            )
            agc = agcp.tile([128, KC, ncols], BF16, tag=f"agc{ncols}",
                            name="agc")
            nc.gpsimd.dma_start(
                agc, outs.ap().rearrange("(kc p) t -> p kc t", p=128))
            state["agc"][key] = agc

        norm_b = {"pending": None}

        def emit_norm_b():
            """Part B of the previous slot's normalize: the broadcast-mult,
            the a2a staging writes, and (on batch boundaries) the collective
            trigger + gather prefetch."""
            if norm_b["pending"] is None:
                return
            b, qt, acc, bcast = norm_b["pending"]
            norm_b["pending"] = None
            onorm = nrm.tile([HD, 512], BF16, tag="onorm", name="onorm")
            nc.vector.tensor_tensor(onorm, acc[0:HD, :], bcast, MULT)
            if b < 3:
                for half in range(2):
                    j = 2 * qt + half
                    nc.sync.dma_start(
                        a2a_in[b].ap()[j * HD:(j + 1) * HD, :],
                        onorm[:, half * 256:(half + 1) * 256])
                if qt == 3:
                    trigger_a2a(a2a_in[b], a2a_out[b], b, BSLICE)
            else:
                # batch 3: owner j holds tokens [j*128,(j+1)*128) per half
                bh = qt // 2
                for c in range(4):
                    j = 4 * (qt % 2) + c
                    nc.sync.dma_start(
                        a2a3_in[bh].ap()[j * HD:(j + 1) * HD, :],
                        onorm[:, c * 128:(c + 1) * 128])
                if qt % 2 == 1:
                    trigger_a2a(a2a3_in[bh], a2a3_out[bh], f"3{bh}", 128)

        def attention_slot(b, qt, fillers):
            """One attention tile (512 q tokens, 16 k tiles as 8 pairs) with
            filler matmuls interleaved so the PE stays dense."""
            q_b, k_b, v_b = qT[b % 2], kT[b % 2], vA[b % 2]
            fill = list(fillers)
            fi = 0
            nfill = len(fill)
            acc = psacc.tile([128, 512], F32, name="acc")
            Pt_t = [None] * 8

            def emit_sc(p):
                sc = pssc.tile([128, 1024], F32, tag="sc", name="sc")
                for h in range(2):
                    kt = 2 * p + h
                    nc.tensor.matmul(
                        sc[:, h * 512:(h + 1) * 512],
                        k_b[:, kt * 128:(kt + 1) * 128],
                        q_b[:, qt * 512:(qt + 1) * 512],
                        start=True, stop=True,
                    )
                Pt = Pp.tile([128, 1024], BF16, tag="P", name="Pt")
                Pt_t[p] = Pt
                nc.scalar.activation(Pt, sc, EXP, scale=SCALE)

            def emit_pv(p):
                Pt = Pt_t[p]
                for h in range(2):
                    kt = 2 * p + h
                    nc.tensor.matmul(
                        acc, v_b[:, kt, :], Pt[:, h * 512:(h + 1) * 512],
                        start=(kt == 0), stop=(kt == 15),
                    )

            # interleave: sc(p+1) ... fillers ... pv(p)
            emit_sc(0)
            for p in range(8):
                if p + 1 < 8:
                    emit_sc(p + 1)
                # spread fillers evenly over the 8 pair slots
                ntake = (nfill * (p + 1)) // 8 - fi
                for _ in range(ntake):
                    fill[fi]()
                    fi += 1
                if p == 2:
                    # previous slot's normalize part B: by now its broadcast
                    # DMA has landed, so the DVE queue won't block on it
                    emit_norm_b()
                emit_pv(p)

            # normalize part A: denominators -> reciprocal -> DRAM bounce ->
            # stride-0 broadcast (no PE involved)
            dnrow = rcp.tile([1, 512], F32, tag="dnrow", name="dnrow")
            nc.vector.tensor_copy(out=dnrow, in_=acc[HD:HD + 1, :])
            dn = rcp.tile([128, 4], F32, tag="dn", name="dn")
            nc.sync.dma_start(dn, dnrow)
            rc = rcp.tile([128, 4], F32, tag="rc", name="rc")
            nc.vector.reciprocal(rc, dn)
            bounce = dnb[qt % 2]
            nc.sync.dma_start(bounce.ap(), rc)
            bcast = nrm.tile([HD, 512], F32, tag="bcast", name="bcast")
            b_ap = bounce.ap()
            bcast_src = bass.AP(
                tensor=b_ap.tensor, offset=b_ap.offset,
                ap=[[0, HD]] + list(b_ap.ap)[1:],
            )
            nc.sync.dma_start(bcast, bcast_src)
            norm_b["pending"] = (b, qt, acc, bcast)

        # ---------- main schedule ----------
        # x prefetch: strict consumption-order queue, 2 steps ahead
        load_x_group(0, sync=True)
        load_x_group(1, sync=True)
        # prologue: qkv(0) dense
        for g in range(4):
            for f in qkv_fillers(0, g):
                f()
            load_x_group(g + 2)

        for b in range(B):
            for qt in range(4):
                step = 4 + b * 4 + qt
                fillers = []
                if b + 1 < B:
                    fillers += qkv_fillers(b + 1, qt)
                if b >= 1 and qt >= 2:
                    # 3 proj chunks of the previous batch per late slot
                    chunks = [0, 1, 2] if qt == 2 else [3, 4, 5]
                    fillers += proj_fillers(
                        b - 1, chunks, BSLICE, (b - 1) * BSLICE)
                attention_slot(b, qt, fillers)
                load_x_group(step + 2)
        emit_norm_b()
        # tail: batch-3 projections (half a overlaps half b's collective)
        for f in proj_fillers("30", [0, 1, 2, 3, 4, 5], 128, 3 * BSLICE):
            f()
        for f in proj_fillers("31", [0, 1, 2, 3, 4, 5], 128, 3 * BSLICE + 128):
            f()

    nc.compile()
    return nc


_NC_CACHE = None


def _get_nc():
    global _NC_CACHE
    if _NC_CACHE is None:
        _NC_CACHE = build_nc()
    return _NC_CACHE


def make_in_maps(x, cos, sin, Wqkv, Wproj, bproj):
    import ml_dtypes

    bf16 = ml_dtypes.bfloat16
    x = np.asarray(x, np.float32)
    cos = np.asarray(cos, np.float32)
    sin = np.asarray(sin, np.float32)
    Wqkv = np.asarray(Wqkv, np.float32)
    Wproj = np.asarray(Wproj, np.float32)
    bproj = np.asarray(bproj, np.float32)

    xT = np.ascontiguousarray(x.reshape(T, C).T.astype(bf16))  # [C, T] bf16
    wprojT = np.ascontiguousarray(Wproj.T.astype(bf16))        # [C_in, C_out]
    s = sin.copy()
    s[:, 0::2] = -sin[:, 0::2]
    cosT = np.ascontiguousarray(cos.T.astype(bf16))            # [HD, S] bf16
    sT = np.ascontiguousarray(s.T.astype(bf16))                # [HD, S] bf16
    bias2 = np.ascontiguousarray(bproj.reshape(KC, 128).T)     # [128, KC]

    in_maps = []
    for h in range(NCORES):
        wh = np.concatenate(
            [
                Wqkv[h * HD:(h + 1) * HD],                 # q rows
                Wqkv[C + h * HD:C + (h + 1) * HD],         # k rows
                Wqkv[2 * C + h * HD:2 * C + (h + 1) * HD], # v rows
            ],
            axis=0,
        )                                                  # [3*HD, C]
        wqkvT_h = np.ascontiguousarray(wh.T.astype(bf16))  # [C, 3*HD]
        in_maps.append({
            "xT": xT,
            "wqkvT": wqkvT_h,
            "wprojT": wprojT,
            "cosT": cosT,
            "sT": sT,
            "bias": bias2,
        })
    return in_maps


def assemble_output(results):
    # batches 0-2: core h's out cols b*256+t <-> token b*S + h*256 + t
    # batch 3: cols 768+t <-> token 3*S + h*128 + t (half a),
    #          cols 896+t <-> token 3*S + 1024 + h*128 + t (half b)
    y = np.empty((T, C), np.float32)
    for h in range(NCORES):
        o = results[h]["out"].T  # [1024, C]
        for b in range(3):
            y[b * S + h * BSLICE:b * S + (h + 1) * BSLICE] = \
                o[b * BSLICE:(b + 1) * BSLICE]
        y[3 * S + h * 128:3 * S + (h + 1) * 128] = o[768:896]
        y[3 * S + 1024 + h * 128:3 * S + 1024 + (h + 1) * 128] = o[896:1024]
    return y.reshape(B, S, C)


def kernel(x, cos, sin, Wqkv, Wproj, bproj, _trace=False, **run_kwargs):
    nc = _get_nc()
    in_maps = make_in_maps(x, cos, sin, Wqkv, Wproj, bproj)
    res = run_bass_kernel_spmd(
        nc, in_maps, core_ids=list(range(NCORES)), trace=_trace, **run_kwargs
    )
    out = assemble_output(res.results)
    kernel.last_results = res
    return out


if __name__ == "__main__":
    nc = build_nc()
    print("built OK, instructions:", len(nc.inst_map))


# revision 13
# speedup vs baseline: 1.2881x; 1.2048x over previous
"""Multi-head attention (B=4, S=2048, C=768, H=8, HD=96) on 8 TRN2 NeuronCores.

Strategy: tensor-parallel by head - one head per core. All TensorEngine
matmuls run bf16 inputs with f32 PSUM accumulation.

Pipeline design (v3):
  - Per-batch AllToAll for batches 0-2; batch 3 is split into two
    half-batch AllToAlls so the tail only waits for a 196KB collective
    plus a 36-matmul projection.
  - Projection for batch b runs as PE "filler" matmuls inside batch b+1's
    attention slots (qt=2,3); batch-3 halves project at the tail.
  - exp runs on [128,1024] PSUM tiles (2 banks), halving ACT instructions.
  - Explicit interleave per attention slot: [sc pair p+1][fillers][pv p]
    so the in-order Tensor queue never waits on the exp dependency.
  - x prefetch is a strict 2-steps-ahead queue over a 3-buffer pool so the
    GpSimd (SWDGE) queue never backs up behind slot-paced WAR waits -
    collective triggers on that queue fire immediately.
  - cos/sin tables bf16 (2x DVE RoPE multiplies); denominator reciprocal
    broadcast via a stride-0 DMA from a DRAM bounce (off the PE).
"""

import numpy as np
from contextlib import ExitStack

import concourse.bass as bass
from concourse import bacc
import concourse.tile as tile
from concourse import mybir
from concourse.bass_utils import run_bass_kernel_spmd

B, S, C, H, HD = 4, 2048, 768, 8, 96
T = B * S            # 8192 tokens
NCORES = 8
TSLICE = T // NCORES  # 1024 tokens per core for the projection
BSLICE = S // NCORES  # 256 tokens per (core, batch)
KC = C // 128        # 6 contraction chunks of 128
F32 = mybir.dt.float32
BF16 = mybir.dt.bfloat16

SCALE = HD ** -0.5
MULT = mybir.AluOpType.mult
ADD = mybir.AluOpType.add
EXP = mybir.ActivationFunctionType.Exp
IDENT = mybir.ActivationFunctionType.Identity

SWAPMASK = []
for i in range(16):
    SWAPMASK += [2 * i + 1, 2 * i]


def build_nc():
    nc = bacc.Bacc(None, num_devices=NCORES)

    xT = nc.declare_dram_parameter("xT", [C, T], BF16, isOutput=False)
    wqkvT = nc.declare_dram_parameter("wqkvT", [C, 3 * HD], BF16, isOutput=False)
    wprojT = nc.declare_dram_parameter("wprojT", [C, C], BF16, isOutput=False)
    cosT = nc.declare_dram_parameter("cosT", [HD, S], BF16, isOutput=False)
    sT = nc.declare_dram_parameter("sT", [HD, S], BF16, isOutput=False)
    biasd = nc.declare_dram_parameter("bias", [128, KC], F32, isOutput=False)
    outd = nc.declare_dram_parameter("out", [C, TSLICE], F32, isOutput=True)

    # batches 0-2: one [C, 256] AllToAll each; batch 3: two [C, 128] halves
    a2a_in = [nc.dram_tensor(f"a2a_in{b}", [C, BSLICE], BF16) for b in range(3)]
    a2a_out = [nc.dram_tensor(f"a2a_out{b}", [C, BSLICE], BF16) for b in range(3)]
    a2a3_in = [nc.dram_tensor(f"a2a3_in{h}", [C, 128], BF16) for h in range(2)]
    a2a3_out = [nc.dram_tensor(f"a2a3_out{h}", [C, 128], BF16) for h in range(2)]
    dnb = [nc.dram_tensor(f"dnb{i}", [1, 512], F32) for i in range(2)]

    with tile.TileContext(nc, num_cores=NCORES) as tc, ExitStack() as ctx:
        const = ctx.enter_context(tc.tile_pool(name="const", bufs=1))
        xtp = ctx.enter_context(tc.tile_pool(name="xtp", bufs=3))
        ropep = ctx.enter_context(tc.tile_pool(name="ropep", bufs=3))
        Pp = ctx.enter_context(tc.tile_pool(name="Pp", bufs=3))
        nrm = ctx.enter_context(tc.tile_pool(name="nrm", bufs=3))
        rcp = ctx.enter_context(tc.tile_pool(name="rcp", bufs=3))
        yp = ctx.enter_context(tc.tile_pool(name="yp", bufs=3))
        agcp = ctx.enter_context(tc.tile_pool(name="agcp", bufs=2))

        # PSUM (8 banks): scores 2x2 + PV acc 2 + qkv/proj 2
        pssc = ctx.enter_context(tc.tile_pool(name="pssc", bufs=2, space="PSUM"))
        psacc = ctx.enter_context(tc.tile_pool(name="psacc", bufs=2, space="PSUM"))
        psqkv = ctx.enter_context(tc.tile_pool(name="psqkv", bufs=2, space="PSUM"))

        # --- constants (wq first so the PE can start ASAP) ---
        wq_sb = const.tile([128, KC, 3 * HD], BF16)
        nc.sync.dma_start(wq_sb, wqkvT.ap().rearrange("(kc p) n -> p kc n", p=128))
        cosT_sb = const.tile([HD, S], BF16)
        nc.sync.dma_start(cosT_sb, cosT.ap())
        sT_sb = const.tile([HD, S], BF16)
        nc.sync.dma_start(sT_sb, sT.ap())
        bias_sb = const.tile([128, KC], F32)
        nc.sync.dma_start(bias_sb, biasd.ap())
        wp_sb = const.tile([128, KC, C], BF16)
        nc.sync.dma_start(wp_sb, wprojT.ap().rearrange("(kc p) n -> p kc n", p=128))

        # persistent ping/pong per-batch q/k (transposed) and token-major v
        qT = [const.tile([HD, S], BF16, name=f"qT{i}") for i in range(2)]
        kT = [const.tile([HD, S], BF16, name=f"kT{i}") for i in range(2)]
        vA = [const.tile([128, 16, 128], BF16, name=f"vA{i}") for i in range(2)]

        # v is produced token-major directly (x-chunk stationary, Wv moving),
        # so vA needs no DMA transposes; column 96 carries ones so PV
        # accumulates the softmax denominator, columns 97..127 stay zero
        for i in range(2):
            nc.vector.memset(vA[i][:, :, HD:HD + 1], 1.0)
            nc.vector.memset(vA[i][:, :, HD + 1:128], 0.0)

        xTv = xT.ap().rearrange("(kc p) t -> p kc t", p=128)  # [128, KC, T]

        # ---------- emission helpers ----------
        state = {"xtc": {}, "agc": {}}
        CONS = [(b, g) for b in range(B) for g in range(4)]  # consumption order

        def load_x_group(idx, sync=False):
            """Prefetch one 512-token x group (consumption-order index)."""
            if idx >= len(CONS):
                return
            b, g = CONS[idx]
            tok0 = b * S + g * 512
            xtc = xtp.tile([128, KC, 512], BF16, tag="xtc", name="xtc")
            eng = nc.sync if sync else nc.gpsimd
            eng.dma_start(xtc, xTv[:, :, tok0:tok0 + 512])
            state["xtc"][(b, g)] = xtc

        def qkv_fillers(b, g):
            """Closures, each emitting one PE matmul of the qkv computation
            for (batch b, 512-token group g); drains are emitted inline by
            the closure that finishes each accumulation."""
            xtc = state["xtc"].pop((b, g))
            seq = slice(g * 512, (g + 1) * 512)
            ps_tiles = {}

            def drain_qk(ps, dstT):
                raw = ropep.tile([HD, 512], BF16, tag="raw", name="raw")
                nc.vector.tensor_copy(out=raw, in_=ps[0:HD, :])
                rot = ropep.tile([HD, 512], BF16, tag="rot", name="rot")
                nc.vector.stream_shuffle(rot, raw, SWAPMASK)
                t1 = ropep.tile([HD, 512], BF16, tag="t1", name="t1")
                nc.vector.tensor_tensor(t1, raw, cosT_sb[:, seq], MULT)
                t2 = ropep.tile([HD, 512], BF16, tag="t2", name="t2")
                nc.vector.tensor_tensor(t2, rot, sT_sb[:, seq], MULT)
                nc.vector.tensor_tensor(dstT[:, seq], t1, t2, ADD)

            def mk(ti, kc):
                def emit():
                    if kc == 0:
                        ps_tiles[ti] = psqkv.tile(
                            [128, 512], F32, tag="qkv", name="qkvps")
                    ps = ps_tiles[ti]
                    nc.tensor.matmul(
                        ps[0:HD, :], wq_sb[:, kc, ti * HD:(ti + 1) * HD],
                        xtc[:, kc, :],
                        start=(kc == 0), stop=(kc == KC - 1),
                    )
                    if kc == KC - 1:
                        drain_qk(ps, (qT if ti == 0 else kT)[b % 2])
                return emit

            def mk_v(c, kc):
                # token-major v: out[tok, d] = x_chunk.T @ WvT - the
                # stationary is the x tile, so vA needs no transpose
                def emit():
                    if c == 0 and kc == 0:
                        ps_tiles["v"] = psqkv.tile(
                            [128, 4, 128], F32, tag="qkv", name="vps")
                    ps = ps_tiles["v"]
                    nc.tensor.matmul(
                        ps[:, c, 0:HD],
                        xtc[:, kc, c * 128:(c + 1) * 128],
                        wq_sb[:, kc, 2 * HD:3 * HD],
                        start=(kc == 0), stop=(kc == KC - 1),
                    )
                    if c == 3 and kc == KC - 1:
                        nc.scalar.copy(
                            out=vA[b % 2][:, 4 * g:4 * g + 4, 0:HD],
                            in_=ps[:, :, 0:HD])
                return emit

            # v first: its drain is on the critical path for the next
            # batch's PV matmuls; RoPE drains can trail
            return ([mk_v(c, kc) for c in range(4) for kc in range(KC)]
                    + [mk(ti, kc) for ti in (0, 1) for kc in range(KC)])

        def proj_fillers(key, chunks, ncols, colbase):
            """Closures for proj chunks (6 matmuls + drain each). `key`
            selects the gathered buffer; output columns [colbase,
            colbase+ncols)."""
            fillers = []
            for ko in chunks:
                def mk(ko):
                    py_ref = {}

                    def emit_mm(kc):
                        if kc == 0:
                            py_ref["py"] = psqkv.tile(
                                [128, 512], F32, tag="qkv", name="pyps")
                        py = py_ref["py"]
                        nc.tensor.matmul(
                            py[:, 0:ncols],
                            wp_sb[:, kc, ko * 128:(ko + 1) * 128],
                            state["agc"][key][:, kc, :],
                            start=(kc == 0), stop=(kc == KC - 1),
                        )
                        if kc == KC - 1:
                            y = yp.tile([128, ncols], F32, tag="y", name="y")
                            nc.scalar.activation(
                                y, py[:, 0:ncols], IDENT,
                                bias=bias_sb[:, ko:ko + 1], scale=1.0,
                            )
                            nc.sync.dma_start(
                                outd.ap()[ko * 128:(ko + 1) * 128,
                                          colbase:colbase + ncols],
                                y)
                    return [lambda kc=kc: emit_mm(kc) for kc in range(KC)]
                fillers += mk(ko)
            return fillers

        def trigger_a2a(ins, outs, key, ncols):
            nc.gpsimd.collective_compute(
                "AllToAll", mybir.AluOpType.bypass,
                replica_groups=[list(range(NCORES))],
                ins=[ins.ap().opt()],
                outs=[outs.ap().opt()],
            )
            agc = agcp.tile([128, KC, ncols], BF16, tag=f"agc{ncols}",
                            name="agc")
            nc.gpsimd.dma_start(
                agc, outs.ap().rearrange("(kc p) t -> p kc t", p=128))
            state["agc"][key] = agc

        norm_b = {"pending": None}

        def emit_norm_b():
            """Part B of the previous slot's normalize: the broadcast-mult,
            the a2a staging writes, and (on batch boundaries) the collective
            trigger + gather prefetch."""
            if norm_b["pending"] is None:
                return
            b, qt, acc, bcast = norm_b["pending"]
            norm_b["pending"] = None
            onorm = nrm.tile([HD, 512], BF16, tag="onorm", name="onorm")
            nc.vector.tensor_tensor(onorm, acc[0:HD, :], bcast, MULT)
            if b < 3:
                for half in range(2):
                    j = 2 * qt + half
                    nc.sync.dma_start(
                        a2a_in[b].ap()[j * HD:(j + 1) * HD, :],
                        onorm[:, half * 256:(half + 1) * 256])
                if qt == 3:
                    trigger_a2a(a2a_in[b], a2a_out[b], b, BSLICE)
            else:
                # batch 3: owner j holds tokens [j*128,(j+1)*128) per half
                bh = qt // 2
                for c in range(4):
                    j = 4 * (qt % 2) + c
                    nc.sync.dma_start(
                        a2a3_in[bh].ap()[j * HD:(j + 1) * HD, :],
                        onorm[:, c * 128:(c + 1) * 128])
                if qt % 2 == 1:
                    trigger_a2a(a2a3_in[bh], a2a3_out[bh], f"3{bh}", 128)

        def attention_slot(b, qt, fillers):
            """One attention tile (512 q tokens, 16 k tiles as 8 pairs) with
            filler matmuls interleaved so the PE stays dense."""
            q_b, k_b, v_b = qT[b % 2], kT[b % 2], vA[b % 2]
            fill = list(fillers)
            fi = 0
            nfill = len(fill)
            acc = psacc.tile([128, 512], F32, name="acc")
            Pt_t = [None] * 8

            def emit_sc(p):
                sc = pssc.tile([128, 1024], F32, tag="sc", name="sc")
                for h in range(2):
                    kt = 2 * p + h
                    nc.tensor.matmul(
                        sc[:, h * 512:(h + 1) * 512],
                        k_b[:, kt * 128:(kt + 1) * 128],
                        q_b[:, qt * 512:(qt + 1) * 512],
                        start=True, stop=True,
                    )
                Pt = Pp.tile([128, 1024], BF16, tag="P", name="Pt")
                Pt_t[p] = Pt
                nc.scalar.activation(Pt, sc, EXP, scale=SCALE)

            def emit_pv(p):
                Pt = Pt_t[p]
                for h in range(2):
                    kt = 2 * p + h
                    nc.tensor.matmul(
                        acc, v_b[:, kt, :], Pt[:, h * 512:(h + 1) * 512],
                        start=(kt == 0), stop=(kt == 15),
                    )

            # interleave: sc(p+1) ... fillers ... pv(p)
            emit_sc(0)
            for p in range(8):
                if p + 1 < 8:
                    emit_sc(p + 1)
                # spread fillers evenly over the 8 pair slots
                ntake = (nfill * (p + 1)) // 8 - fi
                for _ in range(ntake):
                    fill[fi]()
                    fi += 1
                if p == 2:
                    # previous slot's normalize part B: by now its broadcast
                    # DMA has landed, so the DVE queue won't block on it
                    emit_norm_b()
                emit_pv(p)

            # normalize part A: denominators -> reciprocal -> DRAM bounce ->
            # stride-0 broadcast (no PE involved)
            dnrow = rcp.tile([1, 512], F32, tag="dnrow", name="dnrow")
            nc.vector.tensor_copy(out=dnrow, in_=acc[HD:HD + 1, :])
            dn = rcp.tile([128, 4], F32, tag="dn", name="dn")
            nc.sync.dma_start(dn, dnrow)
            rc = rcp.tile([128, 4], F32, tag="rc", name="rc")
            nc.vector.reciprocal(rc, dn)
            bounce = dnb[qt % 2]
            nc.sync.dma_start(bounce.ap(), rc)
            bcast = nrm.tile([HD, 512], F32, tag="bcast", name="bcast")
            b_ap = bounce.ap()
            bcast_src = bass.AP(
                tensor=b_ap.tensor, offset=b_ap.offset,
                ap=[[0, HD]] + list(b_ap.ap)[1:],
            )
            nc.sync.dma_start(bcast, bcast_src)
            norm_b["pending"] = (b, qt, acc, bcast)

        # ---------- main schedule ----------
        # x prefetch: strict consumption-order queue, 2 steps ahead
        load_x_group(0, sync=True)
        load_x_group(1, sync=True)
        # prologue: qkv(0) dense
        for g in range(4):
            for f in qkv_fillers(0, g):
                f()
            load_x_group(g + 2)

        for b in range(B):
            for qt in range(4):
                step = 4 + b * 4 + qt
                fillers = []
                if b + 1 < B:
                    fillers += qkv_fillers(b + 1, qt)
                if b >= 1 and qt >= 2:
                    # 3 proj chunks of the previous batch per late slot
                    chunks = [0, 1, 2] if qt == 2 else [3, 4, 5]
                    fillers += proj_fillers(
                        b - 1, chunks, BSLICE, (b - 1) * BSLICE)
                attention_slot(b, qt, fillers)
                load_x_group(step + 2)
        emit_norm_b()
        # tail: batch-3 projections (half a overlaps half b's collective)
        for f in proj_fillers("30", [0, 1, 2, 3, 4, 5], 128, 3 * BSLICE):
            f()
        for f in proj_fillers("31", [0, 1, 2, 3, 4, 5], 128, 3 * BSLICE + 128):
            f()

    nc.compile()
    return nc


_NC_CACHE = None


def _get_nc():
    global _NC_CACHE
    if _NC_CACHE is None:
        _NC_CACHE = build_nc()
    return _NC_CACHE


def make_in_maps(x, cos, sin, Wqkv, Wproj, bproj):
    import ml_dtypes

    bf16 = ml_dtypes.bfloat16
    x = np.asarray(x, np.float32)
    cos = np.asarray(cos, np.float32)
    sin = np.asarray(sin, np.float32)
    Wqkv = np.asarray(Wqkv, np.float32)
    Wproj = np.asarray(Wproj, np.float32)
    bproj = np.asarray(bproj, np.float32)

    xT = np.ascontiguousarray(x.reshape(T, C).T.astype(bf16))  # [C, T] bf16
    wprojT = np.ascontiguousarray(Wproj.T.astype(bf16))        # [C_in, C_out]
    s = sin.copy()
    s[:, 0::2] = -sin[:, 0::2]
    cosT = np.ascontiguousarray(cos.T.astype(bf16))            # [HD, S] bf16
    sT = np.ascontiguousarray(s.T.astype(bf16))                # [HD, S] bf16
    bias2 = np.ascontiguousarray(bproj.reshape(KC, 128).T)     # [128, KC]

    in_maps = []
    for h in range(NCORES):
        wh = np.concatenate(
            [
                Wqkv[h * HD:(h + 1) * HD],                 # q rows
                Wqkv[C + h * HD:C + (h + 1) * HD],         # k rows
                Wqkv[2 * C + h * HD:2 * C + (h + 1) * HD], # v rows
            ],
            axis=0,
        )                                                  # [3*HD, C]
        wqkvT_h = np.ascontiguousarray(wh.T.astype(bf16))  # [C, 3*HD]
        in_maps.append({
            "xT": xT,
            "wqkvT": wqkvT_h,
            "wprojT": wprojT,
            "cosT": cosT,
            "sT": sT,
            "bias": bias2,
        })
    return in_maps


def assemble_output(results):
    # batches 0-2: core h's out cols b*256+t <-> token b*S + h*256 + t
    # batch 3: cols 768+t <-> token 3*S + h*128 + t (half a),
    #          cols 896+t <-> token 3*S + 1024 + h*128 + t (half b)
    y = np.empty((T, C), np.float32)
    for h in range(NCORES):
        o = results[h]["out"].T  # [1024, C]
        for b in range(3):
            y[b * S + h * BSLICE:b * S + (h + 1) * BSLICE] = \
                o[b * BSLICE:(b + 1) * BSLICE]
        y[3 * S + h * 128:3 * S + (h + 1) * 128] = o[768:896]
        y[3 * S + 1024 + h * 128:3 * S + 1024 + (h + 1) * 128] = o[896:1024]
    return y.reshape(B, S, C)


def kernel(x, cos, sin, Wqkv, Wproj, bproj, _trace=False, **run_kwargs):
    nc = _get_nc()
    in_maps = make_in_maps(x, cos, sin, Wqkv, Wproj, bproj)
    res = run_bass_kernel_spmd(
        nc, in_maps, core_ids=list(range(NCORES)), trace=_trace, **run_kwargs
    )
    out = assemble_output(res.results)
    kernel.last_results = res
    return out


if __name__ == "__main__":
    nc = build_nc()
    print("built OK, instructions:", len(nc.inst_map))


# revision 15
# speedup vs baseline: 1.4548x; 1.1294x over previous
"""Multi-head attention (B=4, S=2048, C=768, H=8, HD=96) on 8 TRN2 NeuronCores.

Strategy: tensor-parallel by head - one head per core. All TensorEngine
matmuls run bf16 inputs with f32 PSUM accumulation.

Pipeline design (v3):
  - Per-batch AllToAll for batches 0-2; batch 3 is split into two
    half-batch AllToAlls so the tail only waits for a 196KB collective
    plus a 36-matmul projection.
  - Projection for batch b runs as PE "filler" matmuls inside batch b+1's
    attention slots (qt=2,3); batch-3 halves project at the tail.
  - exp runs on [128,1024] PSUM tiles (2 banks), halving ACT instructions.
  - Explicit interleave per attention slot: [sc pair p+1][fillers][pv p]
    so the in-order Tensor queue never waits on the exp dependency.
  - x prefetch is a strict 2-steps-ahead queue over a 3-buffer pool so the
    GpSimd (SWDGE) queue never backs up behind slot-paced WAR waits -
    collective triggers on that queue fire immediately.
  - cos/sin tables bf16 (2x DVE RoPE multiplies); denominator reciprocal
    broadcast via a stride-0 DMA from a DRAM bounce (off the PE).
"""

import numpy as np
from contextlib import ExitStack

import concourse.bass as bass
from concourse import bacc
import concourse.tile as tile
from concourse import mybir
from concourse.bass_utils import run_bass_kernel_spmd

B, S, C, H, HD = 4, 2048, 768, 8, 96
T = B * S            # 8192 tokens
NCORES = 8
TSLICE = T // NCORES  # 1024 tokens per core for the projection
BSLICE = S // NCORES  # 256 tokens per (core, batch)
KC = C // 128        # 6 contraction chunks of 128
F32 = mybir.dt.float32
BF16 = mybir.dt.bfloat16

SCALE = HD ** -0.5
MULT = mybir.AluOpType.mult
ADD = mybir.AluOpType.add
EXP = mybir.ActivationFunctionType.Exp
IDENT = mybir.ActivationFunctionType.Identity

SWAPMASK = []
for i in range(16):
    SWAPMASK += [2 * i + 1, 2 * i]


def build_nc():
    nc = bacc.Bacc(None, num_devices=NCORES)

    xT = nc.declare_dram_parameter("xT", [C, T], BF16, isOutput=False)
    wqkvT = nc.declare_dram_parameter("wqkvT", [C, 3 * HD], BF16, isOutput=False)
    wprojT = nc.declare_dram_parameter("wprojT", [C, C], BF16, isOutput=False)
    cosT = nc.declare_dram_parameter("cosT", [HD, S], BF16, isOutput=False)
    sT = nc.declare_dram_parameter("sT", [HD, S], BF16, isOutput=False)
    biasd = nc.declare_dram_parameter("bias", [128, KC], F32, isOutput=False)
    outd = nc.declare_dram_parameter("out", [C, TSLICE], F32, isOutput=True)

    # batches 0-2: one [C, 256] AllToAll each; batch 3: two [C, 128] halves
    a2a_in = [nc.dram_tensor(f"a2a_in{b}", [C, BSLICE], BF16) for b in range(3)]
    a2a_out = [nc.dram_tensor(f"a2a_out{b}", [C, BSLICE], BF16) for b in range(3)]
    a2a3_in = [nc.dram_tensor(f"a2a3_in{h}", [C, 128], BF16) for h in range(2)]
    a2a3_out = [nc.dram_tensor(f"a2a3_out{h}", [C, 128], BF16) for h in range(2)]
    dnb = [nc.dram_tensor(f"dnb{i}", [1, 512], F32) for i in range(2)]

    with tile.TileContext(nc, num_cores=NCORES) as tc, ExitStack() as ctx:
        const = ctx.enter_context(tc.tile_pool(name="const", bufs=1))
        xtp = ctx.enter_context(tc.tile_pool(name="xtp", bufs=3))
        ropep = ctx.enter_context(tc.tile_pool(name="ropep", bufs=3))
        Pp = ctx.enter_context(tc.tile_pool(name="Pp", bufs=3))
        nrm = ctx.enter_context(tc.tile_pool(name="nrm", bufs=3))
        rcp = ctx.enter_context(tc.tile_pool(name="rcp", bufs=3))
        yp = ctx.enter_context(tc.tile_pool(name="yp", bufs=3))
        agcp = ctx.enter_context(tc.tile_pool(name="agcp", bufs=2))

        # PSUM (8 banks): scores 2x2 + PV acc 2 + qkv/proj 2
        pssc = ctx.enter_context(tc.tile_pool(name="pssc", bufs=2, space="PSUM"))
        psacc = ctx.enter_context(tc.tile_pool(name="psacc", bufs=2, space="PSUM"))
        psqkv = ctx.enter_context(tc.tile_pool(name="psqkv", bufs=2, space="PSUM"))

        # --- constants (wq first so the PE can start ASAP) ---
        wq_sb = const.tile([128, KC, 3 * HD], BF16)
        nc.sync.dma_start(wq_sb, wqkvT.ap().rearrange("(kc p) n -> p kc n", p=128))
        cosT_sb = const.tile([HD, S], BF16)
        nc.sync.dma_start(cosT_sb, cosT.ap())
        sT_sb = const.tile([HD, S], BF16)
        nc.sync.dma_start(sT_sb, sT.ap())
        bias_sb = const.tile([128, KC], F32)
        nc.sync.dma_start(bias_sb, biasd.ap())
        wp_sb = const.tile([128, KC, C], BF16)
        nc.sync.dma_start(wp_sb, wprojT.ap().rearrange("(kc p) n -> p kc n", p=128))

        # persistent ping/pong per-batch q/k (transposed) and token-major v
        qT = [const.tile([HD, S], BF16, name=f"qT{i}") for i in range(2)]
        kT = [const.tile([HD, S], BF16, name=f"kT{i}") for i in range(2)]
        vA = [const.tile([128, 16, 128], BF16, name=f"vA{i}") for i in range(2)]

        # v is produced token-major directly (x-chunk stationary, Wv moving),
        # so vA needs no DMA transposes; column 96 carries ones so PV
        # accumulates the softmax denominator, columns 97..127 stay zero
        for i in range(2):
            nc.vector.memset(vA[i][:, :, HD:HD + 1], 1.0)
            nc.vector.memset(vA[i][:, :, HD + 1:128], 0.0)

        xTv = xT.ap().rearrange("(kc p) t -> p kc t", p=128)  # [128, KC, T]

        # ---------- emission helpers ----------
        state = {"xtc": {}, "agc": {}, "a2a_out": {}}
        CONS = [(b, g) for b in range(B) for g in range(4)]  # consumption order

        def load_x_group(idx, sync=False):
            """Prefetch one 512-token x group (consumption-order index)."""
            if idx >= len(CONS):
                return
            b, g = CONS[idx]
            tok0 = b * S + g * 512
            xtc = xtp.tile([128, KC, 512], BF16, tag="xtc", name="xtc")
            eng = nc.sync if sync else nc.gpsimd
            eng.dma_start(xtc, xTv[:, :, tok0:tok0 + 512])
            state["xtc"][(b, g)] = xtc

        def qkv_fillers(b, g):
            """Closures, each emitting one PE matmul of the qkv computation
            for (batch b, 512-token group g); drains are emitted inline by
            the closure that finishes each accumulation."""
            xtc = state["xtc"].pop((b, g))
            seq = slice(g * 512, (g + 1) * 512)
            ps_tiles = {}

            def drain_qk(ps, dstT):
                raw = ropep.tile([HD, 512], BF16, tag="raw", name="raw")
                nc.vector.tensor_copy(out=raw, in_=ps[0:HD, :])
                rot = ropep.tile([HD, 512], BF16, tag="rot", name="rot")
                nc.vector.stream_shuffle(rot, raw, SWAPMASK)
                t1 = ropep.tile([HD, 512], BF16, tag="t1", name="t1")
                nc.vector.tensor_tensor(t1, raw, cosT_sb[:, seq], MULT)
                t2 = ropep.tile([HD, 512], BF16, tag="t2", name="t2")
                nc.vector.tensor_tensor(t2, rot, sT_sb[:, seq], MULT)
                nc.vector.tensor_tensor(dstT[:, seq], t1, t2, ADD)

            def mk(ti, kc):
                def emit():
                    if kc == 0:
                        ps_tiles[ti] = psqkv.tile(
                            [128, 512], F32, tag="qkv", name="qkvps")
                    ps = ps_tiles[ti]
                    nc.tensor.matmul(
                        ps[0:HD, :], wq_sb[:, kc, ti * HD:(ti + 1) * HD],
                        xtc[:, kc, :],
                        start=(kc == 0), stop=(kc == KC - 1),
                    )
                    if kc == KC - 1:
                        drain_qk(ps, (qT if ti == 0 else kT)[b % 2])
                return emit

            def mk_v(c, kc):
                # token-major v: out[tok, d] = x_chunk.T @ WvT - the
                # stationary is the x tile, so vA needs no transpose
                def emit():
                    if c == 0 and kc == 0:
                        ps_tiles["v"] = psqkv.tile(
                            [128, 4, 128], F32, tag="qkv", name="vps")
                    ps = ps_tiles["v"]
                    nc.tensor.matmul(
                        ps[:, c, 0:HD],
                        xtc[:, kc, c * 128:(c + 1) * 128],
                        wq_sb[:, kc, 2 * HD:3 * HD],
                        start=(kc == 0), stop=(kc == KC - 1),
                    )
                    if c == 3 and kc == KC - 1:
                        nc.scalar.copy(
                            out=vA[b % 2][:, 4 * g:4 * g + 4, 0:HD],
                            in_=ps[:, :, 0:HD])
                return emit

            # v first: its drain is on the critical path for the next
            # batch's PV matmuls; RoPE drains can trail
            return ([mk_v(c, kc) for c in range(4) for kc in range(KC)]
                    + [mk(ti, kc) for ti in (0, 1) for kc in range(KC)])

        def proj_fillers(key, chunks, ncols, colbase):
            """Closures for proj chunks (6 matmuls + drain each). `key`
            selects the gathered buffer; output columns [colbase,
            colbase+ncols)."""
            fillers = []
            for ko in chunks:
                def mk(ko):
                    py_ref = {}

                    def emit_mm(kc):
                        if kc == 0:
                            py_ref["py"] = psqkv.tile(
                                [128, 512], F32, tag="qkv", name="pyps")
                        py = py_ref["py"]
                        nc.tensor.matmul(
                            py[:, 0:ncols],
                            wp_sb[:, kc, ko * 128:(ko + 1) * 128],
                            state["agc"][key][:, kc, :],
                            start=(kc == 0), stop=(kc == KC - 1),
                        )
                        if kc == KC - 1:
                            y = yp.tile([128, ncols], F32, tag="y", name="y")
                            nc.scalar.activation(
                                y, py[:, 0:ncols], IDENT,
                                bias=bias_sb[:, ko:ko + 1], scale=1.0,
                            )
                            nc.sync.dma_start(
                                outd.ap()[ko * 128:(ko + 1) * 128,
                                          colbase:colbase + ncols],
                                y)
                    return [lambda kc=kc: emit_mm(kc) for kc in range(KC)]
                fillers += mk(ko)
            return fillers

        def trigger_a2a(ins, outs, key):
            nc.gpsimd.collective_compute(
                "AllToAll", mybir.AluOpType.bypass,
                replica_groups=[list(range(NCORES))],
                ins=[ins.ap().opt()],
                outs=[outs.ap().opt()],
            )
            state["a2a_out"][key] = outs

        def emit_agc(key, ncols):
            """Load the gathered buffer. Emitted well after its collective
            completes so this (GpSimd-queue) DMA's wait never delays later
            collective triggers queued behind it."""
            agc = agcp.tile([128, KC, ncols], BF16, tag=f"agc{ncols}",
                            name="agc")
            nc.gpsimd.dma_start(
                agc,
                state["a2a_out"][key].ap().rearrange("(kc p) t -> p kc t",
                                                     p=128))
            state["agc"][key] = agc

        norm_b = {"pending": None}

        def emit_norm_b():
            """Part B of the previous slot's normalize: the broadcast-mult,
            the a2a staging writes, and (on batch boundaries) the collective
            trigger + gather prefetch."""
            if norm_b["pending"] is None:
                return
            b, qt, acc, bcast = norm_b["pending"]
            norm_b["pending"] = None
            onorm = nrm.tile([HD, 512], BF16, tag="onorm", name="onorm")
            nc.vector.tensor_tensor(onorm, acc[0:HD, :], bcast, MULT)
            if b < 3:
                for half in range(2):
                    j = 2 * qt + half
                    nc.sync.dma_start(
                        a2a_in[b].ap()[j * HD:(j + 1) * HD, :],
                        onorm[:, half * 256:(half + 1) * 256])
                if qt == 3:
                    trigger_a2a(a2a_in[b], a2a_out[b], b)
            else:
                # batch 3: owner j holds tokens [j*128,(j+1)*128) per half
                bh = qt // 2
                for c in range(4):
                    j = 4 * (qt % 2) + c
                    nc.sync.dma_start(
                        a2a3_in[bh].ap()[j * HD:(j + 1) * HD, :],
                        onorm[:, c * 128:(c + 1) * 128])
                if qt % 2 == 1:
                    trigger_a2a(a2a3_in[bh], a2a3_out[bh], f"3{bh}")

        def attention_slot(b, qt, fillers):
            """One attention tile (512 q tokens, 16 k tiles as 8 pairs) with
            filler matmuls interleaved so the PE stays dense."""
            q_b, k_b, v_b = qT[b % 2], kT[b % 2], vA[b % 2]
            fill = list(fillers)
            fi = 0
            nfill = len(fill)
            acc = psacc.tile([128, 512], F32, name="acc")
            Pt_t = [None] * 8

            def emit_sc(p):
                sc = pssc.tile([128, 1024], F32, tag="sc", name="sc")
                for h in range(2):
                    kt = 2 * p + h
                    nc.tensor.matmul(
                        sc[:, h * 512:(h + 1) * 512],
                        k_b[:, kt * 128:(kt + 1) * 128],
                        q_b[:, qt * 512:(qt + 1) * 512],
                        start=True, stop=True,
                    )
                Pt = Pp.tile([128, 1024], BF16, tag="P", name="Pt")
                Pt_t[p] = Pt
                nc.scalar.activation(Pt, sc, EXP, scale=SCALE)

            def emit_pv(p):
                Pt = Pt_t[p]
                for h in range(2):
                    kt = 2 * p + h
                    nc.tensor.matmul(
                        acc, v_b[:, kt, :], Pt[:, h * 512:(h + 1) * 512],
                        start=(kt == 0), stop=(kt == 15),
                    )

            # interleave: sc(p+1) ... fillers ... pv(p)
            emit_sc(0)
            for p in range(8):
                if p + 1 < 8:
                    emit_sc(p + 1)
                # spread fillers evenly over the 8 pair slots
                ntake = (nfill * (p + 1)) // 8 - fi
                for _ in range(ntake):
                    fill[fi]()
                    fi += 1
                if p == 2:
                    # previous slot's normalize part B: by now its broadcast
                    # DMA has landed, so the DVE queue won't block on it
                    emit_norm_b()
                emit_pv(p)

            # normalize part A: denominators -> reciprocal -> DRAM bounce ->
            # stride-0 broadcast (no PE involved)
            dnrow = rcp.tile([1, 512], F32, tag="dnrow", name="dnrow")
            nc.vector.tensor_copy(out=dnrow, in_=acc[HD:HD + 1, :])
            dn = rcp.tile([128, 4], F32, tag="dn", name="dn")
            nc.sync.dma_start(dn, dnrow)
            rc = rcp.tile([128, 4], F32, tag="rc", name="rc")
            nc.vector.reciprocal(rc, dn)
            bounce = dnb[qt % 2]
            nc.sync.dma_start(bounce.ap(), rc)
            bcast = nrm.tile([HD, 512], F32, tag="bcast", name="bcast")
            b_ap = bounce.ap()
            bcast_src = bass.AP(
                tensor=b_ap.tensor, offset=b_ap.offset,
                ap=[[0, HD]] + list(b_ap.ap)[1:],
            )
            nc.sync.dma_start(bcast, bcast_src)
            norm_b["pending"] = (b, qt, acc, bcast)

        # ---------- main schedule ----------
        # x prefetch: strict consumption-order queue, 2 steps ahead
        load_x_group(0, sync=True)
        load_x_group(1, sync=True)
        # prologue: qkv(0) dense
        for g in range(4):
            for f in qkv_fillers(0, g):
                f()
            load_x_group(g + 2)

        # proj(b) runs two batches after b so even a slow collective
        # (entry-barrier skew on the first one) is done before its matmuls
        # hit the in-order Tensor queue
        PROJ_AT = {(2, 2): 0, (2, 3): 0, (3, 0): 1, (3, 1): 1,
                   (3, 2): 2, (3, 3): 2}
        AGC_AT = {(2, 0): (0, BSLICE), (2, 2): (1, BSLICE),
                  (3, 1): (2, BSLICE)}
        for b in range(B):
            for qt in range(4):
                step = 4 + b * 4 + qt
                fillers = []
                if b + 1 < B:
                    fillers += qkv_fillers(b + 1, qt)
                pb = PROJ_AT.get((b, qt))
                if pb is not None:
                    chunks = [0, 1, 2] if qt in (0, 2) else [3, 4, 5]
                    fillers += proj_fillers(pb, chunks, BSLICE, pb * BSLICE)
                attention_slot(b, qt, fillers)
                load_x_group(step + 2)
                if (b, qt) in AGC_AT:
                    emit_agc(*AGC_AT[(b, qt)])
        emit_norm_b()
        # tail: batch-3 projections (half a overlaps half b's collective)
        emit_agc("30", 128)
        for f in proj_fillers("30", [0, 1, 2, 3, 4, 5], 128, 3 * BSLICE):
            f()
        emit_agc("31", 128)
        for f in proj_fillers("31", [0, 1, 2, 3, 4, 5], 128, 3 * BSLICE + 128):
            f()

    nc.compile()
    return nc


_NC_CACHE = None


def _get_nc():
    global _NC_CACHE
    if _NC_CACHE is None:
        _NC_CACHE = build_nc()
    return _NC_CACHE


def make_in_maps(x, cos, sin, Wqkv, Wproj, bproj):
    import ml_dtypes

    bf16 = ml_dtypes.bfloat16
    x = np.asarray(x, np.float32)
    cos = np.asarray(cos, np.float32)
    sin = np.asarray(sin, np.float32)
    Wqkv = np.asarray(Wqkv, np.float32)
    Wproj = np.asarray(Wproj, np.float32)
    bproj = np.asarray(bproj, np.float32)

    xT = np.ascontiguousarray(x.reshape(T, C).T.astype(bf16))  # [C, T] bf16
    wprojT = np.ascontiguousarray(Wproj.T.astype(bf16))        # [C_in, C_out]
    s = sin.copy()
    s[:, 0::2] = -sin[:, 0::2]
    cosT = np.ascontiguousarray(cos.T.astype(bf16))            # [HD, S] bf16
    sT = np.ascontiguousarray(s.T.astype(bf16))                # [HD, S] bf16
    bias2 = np.ascontiguousarray(bproj.reshape(KC, 128).T)     # [128, KC]

    in_maps = []
    for h in range(NCORES):
        wh = np.concatenate(
            [
                Wqkv[h * HD:(h + 1) * HD],                 # q rows
                Wqkv[C + h * HD:C + (h + 1) * HD],         # k rows
                Wqkv[2 * C + h * HD:2 * C + (h + 1) * HD], # v rows
            ],
            axis=0,
        )                                                  # [3*HD, C]
        wqkvT_h = np.ascontiguousarray(wh.T.astype(bf16))  # [C, 3*HD]
        in_maps.append({
            "xT": xT,
            "wqkvT": wqkvT_h,
            "wprojT": wprojT,
            "cosT": cosT,
            "sT": sT,
            "bias": bias2,
        })
    return in_maps


def assemble_output(results):
    # batches 0-2: core h's out cols b*256+t <-> token b*S + h*256 + t
    # batch 3: cols 768+t <-> token 3*S + h*128 + t (half a),
    #          cols 896+t <-> token 3*S + 1024 + h*128 + t (half b)
    y = np.empty((T, C), np.float32)
    for h in range(NCORES):
        o = results[h]["out"].T  # [1024, C]
        for b in range(3):
            y[b * S + h * BSLICE:b * S + (h + 1) * BSLICE] = \
                o[b * BSLICE:(b + 1) * BSLICE]
        y[3 * S + h * 128:3 * S + (h + 1) * 128] = o[768:896]
        y[3 * S + 1024 + h * 128:3 * S + 1024 + (h + 1) * 128] = o[896:1024]
    return y.reshape(B, S, C)


def kernel(x, cos, sin, Wqkv, Wproj, bproj, _trace=False, **run_kwargs):
    nc = _get_nc()
    in_maps = make_in_maps(x, cos, sin, Wqkv, Wproj, bproj)
    res = run_bass_kernel_spmd(
        nc, in_maps, core_ids=list(range(NCORES)), trace=_trace, **run_kwargs
    )
    out = assemble_output(res.results)
    kernel.last_results = res
    return out


if __name__ == "__main__":
    nc = build_nc()
    print("built OK, instructions:", len(nc.inst_map))
